# revision 14
# baseline (speedup 1.0000x reference)
# Trainium2 Bass kernel for nn_DecoderLayer_14663018348828
#
# Reference computation (tokens = B*S, d_model D, ffn F):
#   x = x_enc + x_pos
#   q = elu(x@wq)+1 ; k = elu(x@wk)+1 ; v = x@wv
#   kv = k*v ; kp = cumsum(k, seq) ; kvp = cumsum(kv, seq)
#   attn = q * ((kv + kvp) / kp)
#   h = x + LN(attn; g1, be1)
#   f = relu(h@w1 + b1)^2 @ w2 + b2
#   out = h + LN(f; g2, be2)
#
# Sharding: 8 cores = (batch, seq-half) pairs. Core c owns batch c//2,
# sequence half c%2 (T = S/2 tokens). Everything is token-local except
# the cumsum boundary: odd-half cores add the even half's total k/kv
# sums, exchanged with one pair-wise AllGather of [2, D] per core.
#
# Layout: token-major activations ([token partitions, channel free]).
# x->xT and h->hT transposes run on the PE against an identity matrix.
# The sequence cumsum is a per-128-token-block upper-triangular ones
# matmul plus a rank-1 (ones x offset-row) carry; block offsets come
# from a strict-triangular [NT, NT] matmul over per-block totals with a
# rank-1 carry of the collective result (masked by the core half).

import numpy as np
from contextlib import ExitStack

import concourse.bass as bass
import concourse.bacc as bacc
import concourse.mybir as mybir
from concourse import tile
from concourse.bass_utils import run_bass_kernel_spmd

F32 = mybir.dt.float32
BF16 = mybir.dt.bfloat16
AF = mybir.ActivationFunctionType
ALU = mybir.AluOpType

P = 128


def _chunks(total, step):
    out, o = [], 0
    while o < total:
        c = min(step, total - o)
        out.append((o, c))
        o += c
    return out


class Cfg:
    def __init__(self, B=4, S=4096, D=1024, F=4096, n_cores=8, eps=1e-6,
                 f1_dtype=BF16, w2_dtype=BF16, trivial_affine=True):
        assert n_cores == 2 * B, "pair sharding assumes 2 cores per batch"
        self.B, self.S, self.D, self.F, self.n_cores = B, S, D, F, n_cores
        self.T = (B * S) // n_cores
        assert self.T % P == 0 and D % P == 0 and F % P == 0
        self.NT = self.T // P
        self.KT = D // P
        self.FB = F // P
        self.eps = eps
        self.f1_dtype = f1_dtype
        self.w2_dtype = w2_dtype
        self.trivial_affine = trivial_affine
        self.GT = min(4, self.NT)
        assert self.NT % self.GT == 0
        self.NG = self.NT // self.GT
        self.d_chunks = _chunks(D, 512)


def build_nc(cfg: Cfg):
    nc = bacc.Bacc(None, target_bir_lowering=False,
                   num_devices=cfg.n_cores)
    D, F, T, NT, KT, FB = cfg.D, cfg.F, cfg.T, cfg.NT, cfg.KT, cfg.FB
    GT, NG = cfg.GT, cfg.NG

    xe = nc.dram_tensor("xe", [T, D], F32, kind="ExternalInput")
    xp = nc.dram_tensor("xp", [T, D], F32, kind="ExternalInput")
    wq = nc.dram_tensor("wq", [D, D], F32, kind="ExternalInput")
    wk = nc.dram_tensor("wk", [D, D], F32, kind="ExternalInput")
    wv = nc.dram_tensor("wv", [D, D], F32, kind="ExternalInput")
    w1 = nc.dram_tensor("w1", [D, F], F32, kind="ExternalInput")
    w2 = nc.dram_tensor("w2", [F, D], cfg.w2_dtype, kind="ExternalInput")
    b1c = nc.dram_tensor("b1c", [P, FB], F32, kind="ExternalInput")
    b2B = nc.dram_tensor("b2B", [P, D], F32, kind="ExternalInput")
    uT = nc.dram_tensor("uT", [P, P], F32, kind="ExternalInput")
    onesP1 = nc.dram_tensor("onesP1", [P, 1], F32, kind="ExternalInput")
    ones1P = nc.dram_tensor("ones1P", [1, P], F32, kind="ExternalInput")
    ident = nc.dram_tensor("ident", [P, P], F32, kind="ExternalInput")
    pmask = nc.dram_tensor("pmask", [cfg.n_cores, 1], F32,
                           kind="ExternalInput")
    if not cfg.trivial_affine:
        g1B = nc.dram_tensor("g1B", [P, D], F32, kind="ExternalInput")
        be1B = nc.dram_tensor("be1B", [P, D], F32, kind="ExternalInput")
        g2B = nc.dram_tensor("g2B", [P, D], F32, kind="ExternalInput")
        be2B = nc.dram_tensor("be2B", [P, D], F32, kind="ExternalInput")
    out = nc.dram_tensor("out", [T, D], F32, kind="ExternalOutput")

    cc_groups = [list(range(cfg.n_cores))]

    with tile.TileContext(nc) as tc, ExitStack() as octx:
        consts = octx.enter_context(tc.tile_pool(name="consts", bufs=1))
        dram = octx.enter_context(tc.tile_pool(name="dram", bufs=1,
                                               space="DRAM"))
        stat = octx.enter_context(tc.tile_pool(name="stat", bufs=1))
        ps_tr = octx.enter_context(
            tc.tile_pool(name="ps_tr", bufs=1, space="PSUM"))

        # ---------------- constants ----------------
        uT_sb = consts.tile([P, P], F32)
        nc.gpsimd.dma_start(uT_sb[:], uT[:])
        id_sb = consts.tile([P, P], F32)
        nc.gpsimd.dma_start(id_sb[:], ident[:])
        ones1P_sb = consts.tile([1, P], F32)
        nc.gpsimd.dma_start(ones1P_sb[:], ones1P[:])
        onesP1_sb = consts.tile([P, 1], F32)
        nc.gpsimd.dma_start(onesP1_sb[:], onesP1[:])
        b1c_sb = consts.tile([P, FB], F32)
        nc.gpsimd.dma_start(b1c_sb[:], b1c[:])
        b2B_sb = consts.tile([P, D], F32)
        nc.gpsimd.dma_start(b2B_sb[:], b2B[:])
        pmask_sb = consts.tile([cfg.n_cores, 1], F32)
        nc.gpsimd.dma_start(pmask_sb[:], pmask[:])
        eps_sb = consts.tile([P, 1], F32)
        nc.vector.memset(eps_sb[:], float(cfg.eps))
        affine_sb = {}
        if not cfg.trivial_affine:
            for nm, t in (("g1B", g1B), ("be1B", be1B), ("g2B", g2B),
                          ("be2B", be2B)):
                a = consts.tile([P, D], F32, name=nm + "_sb")
                nc.gpsimd.dma_start(a[:], t[:])
                affine_sb[nm] = a

        x_dr = dram.tile([T, D], F32)
        q_dr = dram.tile([T, D], F32)
        k_dr = dram.tile([T, D], F32)
        kv_dr = dram.tile([T, D], F32)
        cc_in = dram.tile([1, 2 * D], F32)
        cc_out = dram.tile([cfg.n_cores, 2 * D], F32, addr_space="Shared")

        tots_k = stat.tile([NT, D], F32)
        tots_kv = stat.tile([NT, D], F32)
        off_k = stat.tile([1, D], F32)
        off_kv = stat.tile([1, D], F32)

        # =====================================================
        # PASS A
        # =====================================================
        with tc.tile_pool(name="pa", bufs=2) as pa, \
             tc.tile_pool(name="paw", bufs=10) as paw, \
             tc.tile_pool(name="ps_mm", bufs=4, space="PSUM") as ps_mm, \
             tc.tile_pool(name="ps_tot", bufs=2, space="PSUM") as ps_tot:

            gath = pa.tile([cfg.n_cores, 2 * D], F32, tag="gath", bufs=1)
            ccst = pa.tile([1, 2 * D], F32, tag="ccst", bufs=1)
            nc.vector.memset(ccst[:], 0.0)
            for g in range(NG):
                tiles_g = range(g * GT, (g + 1) * GT)
                xt_tiles = {}
                for i in tiles_g:
                    xe_t = pa.tile([P, D], F32, tag="xeA", name=f"xeA{i}")
                    nc.sync.dma_start(xe_t[:], xe[i * P:(i + 1) * P, :])
                    xp_t = pa.tile([P, D], F32, tag="xpA", name=f"xpA{i}")
                    nc.sync.dma_start(xp_t[:], xp[i * P:(i + 1) * P, :])
                    x_i = pa.tile([P, D], F32, tag="xA", name=f"xA{i}")
                    nc.vector.tensor_add(x_i[:], xe_t[:], xp_t[:])
                    nc.sync.dma_start(x_dr[i * P:(i + 1) * P, :], x_i[:])
                    xt_i = pa.tile([P, D], F32, tag="xt", name=f"xt{i}",
                                   bufs=GT + 2)
                    for kb in range(0, KT, 4):
                        nb = min(4, KT - kb)
                        pt = ps_tr.tile([P, nb * P], F32, tag="tr",
                                        name=f"ptr{i}_{kb}")
                        for z in range(nb):
                            nc.tensor.transpose(
                                pt[:, z * P:(z + 1) * P],
                                x_i[:, (kb + z) * P:(kb + z + 1) * P],
                                id_sb[:])
                        nc.vector.tensor_copy(
                            xt_i[:, kb * P:(kb + nb) * P], pt[:])
                    xt_tiles[i] = xt_i

                qkv_sb = {}
                for wname, wdr in (("k", wk), ("v", wv), ("q", wq)):
                    for i in tiles_g:
                        qkv_sb[(wname, i)] = pa.tile(
                            [P, D], F32, tag=f"{wname}A",
                            name=f"{wname}A{i}", bufs=3)
                    for (eo, ec) in cfg.d_chunks:
                        wts = []
                        for kt in range(KT):
                            wt = paw.tile([P, ec], F32, tag="wqkv",
                                          name=f"w{wname}{g}_{eo}_{kt}")
                            nc.sync.dma_start(
                                wt[:], wdr[kt * P:(kt + 1) * P, eo:eo + ec])
                            wts.append(wt)
                        for i in tiles_g:
                            pm = ps_mm.tile([P, ec], F32, tag="pm",
                                            name=f"pm{wname}{i}_{eo}")
                            for kt in range(KT):
                                nc.tensor.matmul(
                                    pm[:],
                                    xt_tiles[i][:, kt * P:(kt + 1) * P],
                                    wts[kt][:], start=(kt == 0),
                                    stop=(kt == KT - 1))
                            dst = qkv_sb[(wname, i)]
                            if wname == "v":
                                nc.vector.scalar_tensor_tensor(
                                    dst[:, eo:eo + ec], pm[:], 0.0,
                                    qkv_sb[("k", i)][:, eo:eo + ec],
                                    op0=ALU.bypass, op1=ALU.mult)
                            else:
                                e_t = pa.tile([P, ec], F32, tag="eluA",
                                              bufs=3,
                                              name=f"elu{wname}{i}_{eo}")
                                nc.scalar.activation(e_t[:], pm[:], AF.Exp)
                                r_t = pa.tile([P, ec], F32, tag="reluA",
                                              bufs=3,
                                              name=f"relu{wname}{i}_{eo}")
                                nc.vector.tensor_scalar(
                                    r_t[:], pm[:], 0.0, 1.0,
                                    op0=ALU.max, op1=ALU.add)
                                nc.vector.tensor_tensor(
                                    dst[:, eo:eo + ec], e_t[:], r_t[:],
                                    op=ALU.min)

                for i in tiles_g:
                    for nm, tots, base in (("k", tots_k, 0),
                                           ("kv", tots_kv, D)):
                        srct = qkv_sb[("k" if nm == "k" else "v", i)]
                        trow = pa.tile([1, D], F32, tag="trow", bufs=3,
                                       name=f"trow{nm}{i}")
                        for (eo, ec) in cfg.d_chunks:
                            ptt = ps_tot.tile([1, ec], F32, tag="ptot",
                                              name=f"ptt{nm}{i}_{eo}")
                            nc.tensor.matmul(ptt[:], onesP1_sb[:],
                                             srct[:, eo:eo + ec],
                                             start=True, stop=True)
                            nc.scalar.copy(trow[:, eo:eo + ec], ptt[:])
                        nc.vector.tensor_tensor(
                            ccst[:, base:base + D], ccst[:, base:base + D],
                            trow[:], op=ALU.add)
                        nc.sync.dma_start(tots[i:i + 1, :], trow[:])
                    nc.sync.dma_start(q_dr[i * P:(i + 1) * P, :],
                                      qkv_sb[("q", i)][:])
                    nc.sync.dma_start(k_dr[i * P:(i + 1) * P, :],
                                      qkv_sb[("k", i)][:])
                    nc.sync.dma_start(kv_dr[i * P:(i + 1) * P, :],
                                      qkv_sb[("v", i)][:])

            # ---------- collective: exchange half totals ----------
            nc.sync.dma_start(cc_in[:], ccst[:])
            nc.gpsimd.collective_compute(
                "AllGather", ALU.bypass, replica_groups=cc_groups,
                ins=[cc_in[:]], outs=[cc_out[:]])
            nc.sync.dma_start(gath[:], cc_out[:])
            # partner totals via one-hot mask matmul (zero on even cores);
            # these seed the running scan-offset rows
            for base, off in ((0, off_k), (D, off_kv)):
                for (eo, ec) in cfg.d_chunks:
                    pco = ps_tot.tile([1, ec], F32, tag="ptot",
                                      name=f"pco{base}_{eo}")
                    nc.tensor.matmul(pco[:], pmask_sb[:],
                                     gath[:, base + eo:base + eo + ec],
                                     start=True, stop=True)
                    nc.scalar.copy(off[:, eo:eo + ec], pco[:])

        # =====================================================
        # PASS B
        # =====================================================
        with tc.tile_pool(name="pb", bufs=2) as pb, \
             tc.tile_pool(name="hpool", bufs=GT + 2) as hpool, \
             tc.tile_pool(name="htp", bufs=1) as htp, \
             tc.tile_pool(name="f1p", bufs=1) as f1p, \
             tc.tile_pool(name="w1p", bufs=2) as w1p, \
             tc.tile_pool(name="w2p", bufs=8) as w2p, \
             tc.tile_pool(name="ps_b", bufs=3, space="PSUM") as ps_b, \
             tc.tile_pool(name="ps_o", bufs=GT, space="PSUM") as ps_o:

            for g in range(NG):
                tiles_g = range(g * GT, (g + 1) * GT)
                h_tiles = {}
                for i in tiles_g:
                    qr = pb.tile([P, D], F32, tag="qB", name=f"qB{i}")
                    nc.sync.dma_start(qr[:], q_dr[i * P:(i + 1) * P, :])
                    kr = pb.tile([P, D], F32, tag="kB", name=f"kB{i}")
                    nc.sync.dma_start(kr[:], k_dr[i * P:(i + 1) * P, :])
                    kvr = pb.tile([P, D], F32, tag="kvB", name=f"kvB{i}")
                    nc.sync.dma_start(kvr[:], kv_dr[i * P:(i + 1) * P, :])
                    x_i = pb.tile([P, D], F32, tag="xB", name=f"xB{i}")
                    nc.sync.dma_start(x_i[:], x_dr[i * P:(i + 1) * P, :])

                    rk = pb.tile([P, D], F32, tag="rkB", name=f"rkB{i}")
                    asum = pb.tile([P, 1], F32, tag="asum", name=f"as{i}")
                    for (eo, ec) in cfg.d_chunks:
                        pck = ps_b.tile([P, ec], F32, tag="misc",
                                        name=f"pck{i}_{eo}")
                        nc.tensor.matmul(pck[:], uT_sb[:], kr[:, eo:eo + ec],
                                         start=True, stop=False)
                        nc.tensor.matmul(pck[:], ones1P_sb[:],
                                         off_k[:, eo:eo + ec],
                                         start=False, stop=True)
                        nc.vector.reciprocal(rk[:, eo:eo + ec], pck[:])
                        pckv = ps_b.tile([P, ec], F32, tag="misc",
                                         name=f"pckv{i}_{eo}")
                        nc.tensor.matmul(pckv[:], uT_sb[:],
                                         kvr[:, eo:eo + ec],
                                         start=True, stop=False)
                        nc.tensor.matmul(pckv[:], ones1P_sb[:],
                                         off_kv[:, eo:eo + ec],
                                         start=False, stop=True)
                        nc.vector.scalar_tensor_tensor(
                            kvr[:, eo:eo + ec], kvr[:, eo:eo + ec], 0.0,
                            pckv[:], op0=ALU.bypass, op1=ALU.add)
                    # advance the running offset rows for the next tile
                    if i < NT - 1:
                        for tots, off, nm in ((tots_k, off_k, "k"),
                                              (tots_kv, off_kv, "kv")):
                            trB = pb.tile([1, D], F32, tag="trB", bufs=2,
                                          name=f"trB{nm}{i}")
                            nc.sync.dma_start(trB[:], tots[i:i + 1, :])
                            nc.vector.tensor_tensor(off[:], off[:], trB[:],
                                                    op=ALU.add)
                    nc.vector.tensor_tensor(kvr[:], kvr[:], rk[:],
                                            op=ALU.mult)
                    nc.vector.scalar_tensor_tensor(
                        qr[:], qr[:], 0.0, kvr[:], op0=ALU.bypass,
                        op1=ALU.mult, accum_out=asum[:])

                    # ---- LN1 ----
                    sq = pb.tile([P, D], F32, tag="sqB", name=f"sqB{i}",
                                 bufs=1)
                    s2 = pb.tile([P, 1], F32, tag="s2", name=f"s2_{i}")
                    nc.scalar.activation(sq[:], qr[:], AF.Square,
                                         accum_out=s2[:])
                    mean = pb.tile([P, 1], F32, tag="mean", name=f"mn{i}")
                    nc.vector.tensor_scalar(mean[:], asum[:], 1.0 / D, None,
                                            op0=ALU.mult)
                    m2 = pb.tile([P, 1], F32, tag="m2", name=f"m2_{i}")
                    nc.vector.tensor_tensor(m2[:], mean[:], mean[:],
                                            op=ALU.mult)
                    var = pb.tile([P, 1], F32, tag="var", name=f"vr{i}")
                    nc.vector.scalar_tensor_tensor(
                        var[:], s2[:], 1.0 / D, m2[:], op0=ALU.mult,
                        op1=ALU.subtract)
                    std = pb.tile([P, 1], F32, tag="std", name=f"sd{i}")
                    nc.scalar.activation(std[:], var[:], AF.Sqrt,
                                         bias=eps_sb[:])
                    inv = pb.tile([P, 1], F32, tag="inv", name=f"iv{i}")
                    nc.vector.reciprocal(inv[:], std[:])
                    nmi = pb.tile([P, 1], F32, tag="nmi", name=f"nm{i}")
                    nc.vector.scalar_tensor_tensor(
                        nmi[:], mean[:], -1.0, inv[:], op0=ALU.mult,
                        op1=ALU.mult)
                    nc.scalar.activation(qr[:], qr[:], AF.Identity,
                                         bias=nmi[:], scale=inv[:])
                    if not cfg.trivial_affine:
                        nc.vector.tensor_tensor(qr[:], qr[:],
                                                affine_sb["g1B"][:],
                                                op=ALU.mult)
                        nc.vector.tensor_tensor(qr[:], qr[:],
                                                affine_sb["be1B"][:],
                                                op=ALU.add)
                    h_i = hpool.tile([P, D], F32, tag="h", name=f"h{i}")
                    nc.vector.tensor_tensor(h_i[:], x_i[:], qr[:],
                                            op=ALU.add)
                    h_tiles[i] = h_i

                # ---- h -> hT ----
                TG = GT * P
                ht = htp.tile([P, KT * TG], F32, tag="ht", name=f"ht{g}")
                for kt in range(KT):
                    for jb in range(0, GT, 4):
                        nb = min(4, GT - jb)
                        pt = ps_tr.tile([P, nb * P], F32, tag="tr",
                                        name=f"ptrh{g}_{kt}_{jb}")
                        for z in range(nb):
                            i = g * GT + jb + z
                            nc.tensor.transpose(
                                pt[:, z * P:(z + 1) * P],
                                h_tiles[i][:, kt * P:(kt + 1) * P], id_sb[:])
                        nc.scalar.copy(
                            ht[:, kt * TG + jb * P: kt * TG + (jb + nb) * P],
                            pt[:])

                # ---- FFN1 ----
                f1t = f1p.tile([P, FB * TG], cfg.f1_dtype, tag="f1t",
                               name=f"f1t{g}")
                w1r = w1.rearrange("(kt p) f -> p kt f", p=P)
                for fb in range(FB):
                    w1t = w1p.tile([P, KT * P], F32, tag="w1t",
                                   name=f"w1t{g}_{fb}")
                    nc.sync.dma_start(
                        w1t[:].rearrange("p (kt f) -> p kt f", kt=KT),
                        w1r[:, :, fb * P:(fb + 1) * P])
                    pf = ps_b.tile([P, TG], F32, tag="misc",
                                   name=f"pf{g}_{fb}")
                    for kt in range(KT):
                        nc.tensor.matmul(
                            pf[:],
                            w1t[:, kt * P:(kt + 1) * P],
                            ht[:, kt * TG:(kt + 1) * TG],
                            start=(kt == 0), stop=(kt == KT - 1))
                    r_f = pb.tile([P, TG], F32, tag="rf", name=f"rf{g}_{fb}")
                    nc.scalar.activation(r_f[:], pf[:], AF.Relu,
                                         bias=b1c_sb[:, fb:fb + 1])
                    if fb % 2 == 0:
                        nc.scalar.square(f1t[:, fb * TG:(fb + 1) * TG],
                                         r_f[:])
                    else:
                        nc.vector.tensor_tensor(
                            f1t[:, fb * TG:(fb + 1) * TG], r_f[:], r_f[:],
                            op=ALU.mult)

                # ---- FFN2 ----
                f2_tiles = {}
                for i in tiles_g:
                    f2_tiles[i] = pb.tile([P, D], F32, tag="f2",
                                          name=f"f2_{i}", bufs=GT)
                f2sum = {i: [] for i in tiles_g}
                for (eo, ec) in cfg.d_chunks:
                    pos = {}
                    for i in tiles_g:
                        pos[i] = ps_o.tile([P, ec], F32, tag="po",
                                           name=f"pq{i}_{eo}")
                    for fb in range(FB):
                        w2t = w2p.tile([P, ec], cfg.w2_dtype, tag="w2t",
                                       name=f"w2t{g}_{eo}_{fb}")
                        nc.sync.dma_start(
                            w2t[:], w2[fb * P:(fb + 1) * P, eo:eo + ec])
                        for j, i in enumerate(tiles_g):
                            nc.tensor.matmul(
                                pos[i][:],
                                f1t[:, fb * TG + j * P:fb * TG + (j + 1) * P],
                                w2t[:], start=(fb == 0), stop=(fb == FB - 1))
                    for i in tiles_g:
                        acc = pb.tile([P, 1], F32, tag="f2s",
                                      name=f"f2s{i}_{eo}", bufs=2 * GT)
                        nc.vector.scalar_tensor_tensor(
                            f2_tiles[i][:, eo:eo + ec], pos[i][:], 0.0,
                            b2B_sb[:, eo:eo + ec], op0=ALU.bypass,
                            op1=ALU.add, accum_out=acc[:])
                        f2sum[i].append(acc)

                # ---- LN2 + residual ----
                for i in tiles_g:
                    f2_i = f2_tiles[i]
                    asum2 = pb.tile([P, 1], F32, tag="asum2",
                                    name=f"as2_{i}")
                    accs = f2sum[i]
                    if len(accs) == 1:
                        nc.vector.tensor_copy(asum2[:], accs[0][:])
                    else:
                        nc.vector.tensor_tensor(asum2[:], accs[0][:],
                                                accs[1][:], op=ALU.add)
                        for a in accs[2:]:
                            nc.vector.tensor_tensor(asum2[:], asum2[:],
                                                    a[:], op=ALU.add)
                    sq2 = pb.tile([P, D], F32, tag="sqB", name=f"sq2_{i}",
                                  bufs=1)
                    s22 = pb.tile([P, 1], F32, tag="s22", name=f"s22_{i}")
                    nc.scalar.activation(sq2[:], f2_i[:], AF.Square,
                                         accum_out=s22[:])
                    mean2 = pb.tile([P, 1], F32, tag="mean2",
                                    name=f"mn2_{i}")
                    nc.vector.tensor_scalar(mean2[:], asum2[:], 1.0 / D,
                                            None, op0=ALU.mult)
                    m22 = pb.tile([P, 1], F32, tag="m22", name=f"m22_{i}")
                    nc.vector.tensor_tensor(m22[:], mean2[:], mean2[:],
                                            op=ALU.mult)
                    var2 = pb.tile([P, 1], F32, tag="var2", name=f"vr2_{i}")
                    nc.vector.scalar_tensor_tensor(
                        var2[:], s22[:], 1.0 / D, m22[:], op0=ALU.mult,
                        op1=ALU.subtract)
                    std2 = pb.tile([P, 1], F32, tag="std2", name=f"sd2_{i}")
                    nc.scalar.activation(std2[:], var2[:], AF.Sqrt,
                                         bias=eps_sb[:])
                    inv2 = pb.tile([P, 1], F32, tag="inv2", name=f"iv2_{i}")
                    nc.vector.reciprocal(inv2[:], std2[:])
                    nmi2 = pb.tile([P, 1], F32, tag="nmi2", name=f"nm2_{i}")
                    nc.vector.scalar_tensor_tensor(
                        nmi2[:], mean2[:], -1.0, inv2[:], op0=ALU.mult,
                        op1=ALU.mult)
                    nc.scalar.activation(f2_i[:], f2_i[:], AF.Identity,
                                         bias=nmi2[:], scale=inv2[:])
                    if not cfg.trivial_affine:
                        nc.vector.tensor_tensor(f2_i[:], f2_i[:],
                                                affine_sb["g2B"][:],
                                                op=ALU.mult)
                        nc.vector.tensor_tensor(f2_i[:], f2_i[:],
                                                affine_sb["be2B"][:],
                                                op=ALU.add)
                    o_i = pb.tile([P, D], F32, tag="oB", name=f"oB{i}")
                    nc.vector.tensor_tensor(o_i[:], h_tiles[i][:], f2_i[:],
                                            op=ALU.add)
                    nc.sync.dma_start(out[i * P:(i + 1) * P, :], o_i[:])

    nc.finalize()
    return nc


# ------------------------------------------------------------------
# host wrapper
# ------------------------------------------------------------------

def _pmask(cfg, core):
    m = np.zeros((cfg.n_cores, 1), np.float32)
    if core % 2 == 1:
        m[core - 1, 0] = 1.0
    return m


def _prep_core_inputs(cfg: Cfg, inputs, core):
    np_bf16 = mybir.dt.np(BF16)
    b = core // 2
    half = core % 2
    T, D, NT = cfg.T, cfg.D, cfg.NT
    f32 = np.float32

    def a(x):
        return np.ascontiguousarray(np.asarray(x, dtype=f32))

    b1 = np.asarray(inputs["b1"], dtype=f32)
    b2 = np.asarray(inputs["b2"], dtype=f32)
    w2 = np.asarray(inputs["w2"], dtype=f32)
    if cfg.w2_dtype == BF16:
        w2 = w2.astype(np_bf16)
    m = {
        "xe": a(inputs["x_enc"][b, half * T:(half + 1) * T, :]),
        "xp": a(inputs["x_pos"][b, half * T:(half + 1) * T, :]),
        "wq": a(inputs["wq"]),
        "wk": a(inputs["wk"]),
        "wv": a(inputs["wv"]),
        "w1": a(inputs["w1"]),
        "w2": np.ascontiguousarray(w2),
        "b1c": np.ascontiguousarray(b1.reshape(cfg.FB, P).T),
        "b2B": np.ascontiguousarray(np.broadcast_to(b2, (P, D)).copy()),
        "uT": np.triu(np.ones((P, P), f32)),
        "onesP1": np.ones((P, 1), f32),
        "ones1P": np.ones((1, P), f32),
        "ident": np.eye(P, dtype=f32),
        "pmask": _pmask(cfg, core),
    }
    if not cfg.trivial_affine:
        for nm, key in (("g1B", "g1"), ("be1B", "be1"), ("g2B", "g2"),
                        ("be2B", "be2")):
            m[nm] = np.ascontiguousarray(
                np.broadcast_to(np.asarray(inputs[key], f32), (P, D)).copy())
    return m


def run_kernel(inputs, cfg=None, trace=False):
    if cfg is None:
        cfg = Cfg()
    trivial = (np.all(np.asarray(inputs["g1"]) == 1.0)
               and np.all(np.asarray(inputs["be1"]) == 0.0)
               and np.all(np.asarray(inputs["g2"]) == 1.0)
               and np.all(np.asarray(inputs["be2"]) == 0.0))
    cfg.trivial_affine = bool(trivial)
    nc = build_nc(cfg)
    in_maps = [_prep_core_inputs(cfg, inputs, c) for c in range(cfg.n_cores)]
    res = run_bass_kernel_spmd(nc, in_maps, core_ids=list(range(cfg.n_cores)),
                               trace=trace)
    out = np.empty((cfg.B, cfg.S, cfg.D), np.float32)
    T = cfg.T
    for c in range(cfg.n_cores):
        out[c // 2, (c % 2) * T:((c % 2) + 1) * T, :] = res.results[c]["out"]
    return out, res


def kernel(**inputs):
    out, _ = run_kernel(inputs)
    return out


# revision 15
# speedup vs baseline: 2.1267x; 2.1267x over previous
# Trainium2 Bass kernel for nn_DecoderLayer_14663018348828
#
# Reference computation (tokens = B*S, d_model D, ffn F):
#   x = x_enc + x_pos
#   q = elu(x@wq)+1 ; k = elu(x@wk)+1 ; v = x@wv
#   kv = k*v ; kp = cumsum(k, seq) ; kvp = cumsum(kv, seq)
#   attn = q * ((kv + kvp) / kp)
#   h = x + LN(attn; g1, be1)
#   f = relu(h@w1 + b1)^2 @ w2 + b2
#   out = h + LN(f; g2, be2)
#
# Sharding: 8 cores = (batch, seq-half) pairs. Core c owns batch c//2,
# sequence half c%2 (T = S/2 tokens). Everything is token-local except
# the cumsum boundary: odd-half cores add the even half's total k/kv
# sums, exchanged with one pair-wise AllGather of [2, D] per core.
#
# Layout: token-major activations ([token partitions, channel free]).
# x->xT and h->hT transposes run on the PE against an identity matrix.
# The sequence cumsum is a per-128-token-block upper-triangular ones
# matmul plus a rank-1 (ones x offset-row) carry; block offsets come
# from a strict-triangular [NT, NT] matmul over per-block totals with a
# rank-1 carry of the collective result (masked by the core half).

import numpy as np
from contextlib import ExitStack

import concourse.bass as bass
import concourse.bacc as bacc
import concourse.mybir as mybir
from concourse import tile
from concourse.bass_utils import run_bass_kernel_spmd

F32 = mybir.dt.float32
BF16 = mybir.dt.bfloat16
AF = mybir.ActivationFunctionType
ALU = mybir.AluOpType

P = 128


def _chunks(total, step):
    out, o = [], 0
    while o < total:
        c = min(step, total - o)
        out.append((o, c))
        o += c
    return out


class Cfg:
    def __init__(self, B=4, S=4096, D=1024, F=4096, n_cores=8, eps=1e-6,
                 f1_dtype=BF16, w2_dtype=BF16, mm_dtype=BF16,
                 trivial_affine=True):
        assert n_cores == 2 * B, "pair sharding assumes 2 cores per batch"
        self.B, self.S, self.D, self.F, self.n_cores = B, S, D, F, n_cores
        self.T = (B * S) // n_cores
        assert self.T % P == 0 and D % P == 0 and F % P == 0
        self.NT = self.T // P
        self.KT = D // P
        self.FB = F // P
        self.eps = eps
        self.f1_dtype = f1_dtype
        self.w2_dtype = w2_dtype
        self.mm_dtype = mm_dtype
        self.trivial_affine = trivial_affine
        self.GT = min(4, self.NT)
        assert self.NT % self.GT == 0
        self.NG = self.NT // self.GT
        self.d_chunks = _chunks(D, 512)


def build_nc(cfg: Cfg):
    nc = bacc.Bacc(None, target_bir_lowering=False,
                   num_devices=cfg.n_cores)
    D, F, T, NT, KT, FB = cfg.D, cfg.F, cfg.T, cfg.NT, cfg.KT, cfg.FB
    GT, NG = cfg.GT, cfg.NG

    xe = nc.dram_tensor("xe", [T, D], F32, kind="ExternalInput")
    xp = nc.dram_tensor("xp", [T, D], F32, kind="ExternalInput")
    MMDT = cfg.mm_dtype
    wq = nc.dram_tensor("wq", [D, D], MMDT, kind="ExternalInput")
    wk = nc.dram_tensor("wk", [D, D], MMDT, kind="ExternalInput")
    wv = nc.dram_tensor("wv", [D, D], MMDT, kind="ExternalInput")
    w1 = nc.dram_tensor("w1", [D, F], MMDT, kind="ExternalInput")
    w2 = nc.dram_tensor("w2", [F, D], cfg.w2_dtype, kind="ExternalInput")
    b1c = nc.dram_tensor("b1c", [P, FB], F32, kind="ExternalInput")
    b2B = nc.dram_tensor("b2B", [P, D], F32, kind="ExternalInput")
    uT = nc.dram_tensor("uT", [P, P], MMDT, kind="ExternalInput")
    onesP1 = nc.dram_tensor("onesP1", [P, 1], MMDT, kind="ExternalInput")
    ones1P = nc.dram_tensor("ones1P", [1, P], MMDT, kind="ExternalInput")
    ident = nc.dram_tensor("ident", [P, P], F32, kind="ExternalInput")
    pmask = nc.dram_tensor("pmask", [cfg.n_cores, 1], F32,
                           kind="ExternalInput")
    if not cfg.trivial_affine:
        g1B = nc.dram_tensor("g1B", [P, D], F32, kind="ExternalInput")
        be1B = nc.dram_tensor("be1B", [P, D], F32, kind="ExternalInput")
        g2B = nc.dram_tensor("g2B", [P, D], F32, kind="ExternalInput")
        be2B = nc.dram_tensor("be2B", [P, D], F32, kind="ExternalInput")
    out = nc.dram_tensor("out", [T, D], F32, kind="ExternalOutput")

    cc_groups = [list(range(cfg.n_cores))]

    with tile.TileContext(nc) as tc, ExitStack() as octx:
        consts = octx.enter_context(tc.tile_pool(name="consts", bufs=1))
        dram = octx.enter_context(tc.tile_pool(name="dram", bufs=1,
                                               space="DRAM"))
        stat = octx.enter_context(tc.tile_pool(name="stat", bufs=1))
        ps_tr = octx.enter_context(
            tc.tile_pool(name="ps_tr", bufs=1, space="PSUM"))

        # ---------------- constants ----------------
        uT_sb = consts.tile([P, P], MMDT)
        nc.gpsimd.dma_start(uT_sb[:], uT[:])
        id_sb = consts.tile([P, P], F32)
        nc.gpsimd.dma_start(id_sb[:], ident[:])
        ones1P_sb = consts.tile([1, P], MMDT)
        nc.gpsimd.dma_start(ones1P_sb[:], ones1P[:])
        onesP1_sb = consts.tile([P, 1], MMDT)
        nc.gpsimd.dma_start(onesP1_sb[:], onesP1[:])
        b1c_sb = consts.tile([P, FB], F32)
        nc.gpsimd.dma_start(b1c_sb[:], b1c[:])
        b2B_sb = consts.tile([P, D], F32)
        nc.gpsimd.dma_start(b2B_sb[:], b2B[:])
        pmask_sb = consts.tile([cfg.n_cores, 1], F32)
        nc.gpsimd.dma_start(pmask_sb[:], pmask[:])
        eps_sb = consts.tile([P, 1], F32)
        nc.vector.memset(eps_sb[:], float(cfg.eps))
        affine_sb = {}
        if not cfg.trivial_affine:
            for nm, t in (("g1B", g1B), ("be1B", be1B), ("g2B", g2B),
                          ("be2B", be2B)):
                a = consts.tile([P, D], F32, name=nm + "_sb")
                nc.gpsimd.dma_start(a[:], t[:])
                affine_sb[nm] = a

        x_dr = dram.tile([T, D], F32)
        q_dr = dram.tile([T, D], F32)
        k_dr = dram.tile([T, D], MMDT)
        kv_dr = dram.tile([T, D], MMDT)
        cc_in = dram.tile([1, 2 * D], F32)
        cc_out = dram.tile([cfg.n_cores, 2 * D], F32, addr_space="Shared")

        tots_k = stat.tile([NT, D], F32)
        tots_kv = stat.tile([NT, D], F32)
        off_k = stat.tile([1, D], F32)
        off_kv = stat.tile([1, D], F32)
        offb_k = stat.tile([1, D], MMDT)
        offb_kv = stat.tile([1, D], MMDT)

        # =====================================================
        # PASS A
        # =====================================================
        with tc.tile_pool(name="pa", bufs=2) as pa, \
             tc.tile_pool(name="paw", bufs=10) as paw, \
             tc.tile_pool(name="ps_mm", bufs=4, space="PSUM") as ps_mm, \
             tc.tile_pool(name="ps_tot", bufs=2, space="PSUM") as ps_tot:

            gath = pa.tile([cfg.n_cores, 2 * D], F32, tag="gath", bufs=1)
            ccst = pa.tile([1, 2 * D], F32, tag="ccst", bufs=1)
            nc.vector.memset(ccst[:], 0.0)
            for g in range(NG):
                tiles_g = range(g * GT, (g + 1) * GT)
                xt_tiles = {}
                for i in tiles_g:
                    xe_t = pa.tile([P, D], F32, tag="xeA", name=f"xeA{i}")
                    nc.sync.dma_start(xe_t[:], xe[i * P:(i + 1) * P, :])
                    xp_t = pa.tile([P, D], F32, tag="xpA", name=f"xpA{i}")
                    nc.sync.dma_start(xp_t[:], xp[i * P:(i + 1) * P, :])
                    x_i = pa.tile([P, D], F32, tag="xA", name=f"xA{i}")
                    nc.vector.tensor_add(x_i[:], xe_t[:], xp_t[:])
                    nc.sync.dma_start(x_dr[i * P:(i + 1) * P, :], x_i[:])
                    xt_i = pa.tile([P, D], MMDT, tag="xt", name=f"xt{i}",
                                   bufs=GT + 2)
                    for kb in range(0, KT, 4):
                        nb = min(4, KT - kb)
                        pt = ps_tr.tile([P, nb * P], F32, tag="tr",
                                        name=f"ptr{i}_{kb}")
                        for z in range(nb):
                            nc.tensor.transpose(
                                pt[:, z * P:(z + 1) * P],
                                x_i[:, (kb + z) * P:(kb + z + 1) * P],
                                id_sb[:])
                        nc.vector.tensor_copy(
                            xt_i[:, kb * P:(kb + nb) * P], pt[:])
                    xt_tiles[i] = xt_i

                qkv_sb = {}
                for wname, wdr in (("k", wk), ("v", wv), ("q", wq)):
                    wdt = F32 if wname == "q" else MMDT
                    for i in tiles_g:
                        qkv_sb[(wname, i)] = pa.tile(
                            [P, D], wdt, tag=f"{wname}A",
                            name=f"{wname}A{i}", bufs=3)
                    for (eo, ec) in cfg.d_chunks:
                        wts = []
                        for kt in range(KT):
                            wt = paw.tile([P, ec], MMDT, tag="wqkv",
                                          name=f"w{wname}{g}_{eo}_{kt}")
                            nc.sync.dma_start(
                                wt[:], wdr[kt * P:(kt + 1) * P, eo:eo + ec])
                            wts.append(wt)
                        for i in tiles_g:
                            pm = ps_mm.tile([P, ec], F32, tag="pm",
                                            name=f"pm{wname}{i}_{eo}")
                            for kt in range(KT):
                                nc.tensor.matmul(
                                    pm[:],
                                    xt_tiles[i][:, kt * P:(kt + 1) * P],
                                    wts[kt][:], start=(kt == 0),
                                    stop=(kt == KT - 1))
                            dst = qkv_sb[(wname, i)]
                            if wname == "v":
                                nc.vector.scalar_tensor_tensor(
                                    dst[:, eo:eo + ec], pm[:], 0.0,
                                    qkv_sb[("k", i)][:, eo:eo + ec],
                                    op0=ALU.bypass, op1=ALU.mult)
                            else:
                                e_t = pa.tile([P, ec], F32, tag="eluA",
                                              bufs=3,
                                              name=f"elu{wname}{i}_{eo}")
                                nc.scalar.activation(e_t[:], pm[:], AF.Exp)
                                r_t = pa.tile([P, ec], F32, tag="reluA",
                                              bufs=3,
                                              name=f"relu{wname}{i}_{eo}")
                                nc.vector.tensor_scalar(
                                    r_t[:], pm[:], 0.0, 1.0,
                                    op0=ALU.max, op1=ALU.add)
                                nc.vector.tensor_tensor(
                                    dst[:, eo:eo + ec], e_t[:], r_t[:],
                                    op=ALU.min)

                for i in tiles_g:
                    for nm, tots, base in (("k", tots_k, 0),
                                           ("kv", tots_kv, D)):
                        srct = qkv_sb[("k" if nm == "k" else "v", i)]
                        trow = pa.tile([1, D], F32, tag="trow", bufs=3,
                                       name=f"trow{nm}{i}")
                        for (eo, ec) in cfg.d_chunks:
                            ptt = ps_tot.tile([1, ec], F32, tag="ptot",
                                              name=f"ptt{nm}{i}_{eo}")
                            nc.tensor.matmul(ptt[:], onesP1_sb[:],
                                             srct[:, eo:eo + ec],
                                             start=True, stop=True)
                            nc.scalar.copy(trow[:, eo:eo + ec], ptt[:])
                        nc.vector.tensor_tensor(
                            ccst[:, base:base + D], ccst[:, base:base + D],
                            trow[:], op=ALU.add)
                        nc.sync.dma_start(tots[i:i + 1, :], trow[:])
                    nc.sync.dma_start(q_dr[i * P:(i + 1) * P, :],
                                      qkv_sb[("q", i)][:])
                    nc.sync.dma_start(k_dr[i * P:(i + 1) * P, :],
                                      qkv_sb[("k", i)][:])
                    nc.sync.dma_start(kv_dr[i * P:(i + 1) * P, :],
                                      qkv_sb[("v", i)][:])

            # ---------- collective: exchange half totals ----------
            nc.sync.dma_start(cc_in[:], ccst[:])
            nc.gpsimd.collective_compute(
                "AllGather", ALU.bypass, replica_groups=cc_groups,
                ins=[cc_in[:]], outs=[cc_out[:]])
            nc.sync.dma_start(gath[:], cc_out[:])
            # partner totals via one-hot mask matmul (zero on even cores);
            # these seed the running scan-offset rows
            for base, off, offb in ((0, off_k, offb_k), (D, off_kv, offb_kv)):
                for (eo, ec) in cfg.d_chunks:
                    pco = ps_tot.tile([1, ec], F32, tag="ptot",
                                      name=f"pco{base}_{eo}")
                    nc.tensor.matmul(pco[:], pmask_sb[:],
                                     gath[:, base + eo:base + eo + ec],
                                     start=True, stop=True)
                    nc.scalar.copy(off[:, eo:eo + ec], pco[:])
                nc.vector.tensor_copy(offb[:], off[:])

        # =====================================================
        # PASS B
        # =====================================================
        with tc.tile_pool(name="pb", bufs=2) as pb, \
             tc.tile_pool(name="hpool", bufs=GT + 2) as hpool, \
             tc.tile_pool(name="htp", bufs=1) as htp, \
             tc.tile_pool(name="f1p", bufs=1) as f1p, \
             tc.tile_pool(name="w1p", bufs=2) as w1p, \
             tc.tile_pool(name="w2p", bufs=8) as w2p, \
             tc.tile_pool(name="ps_b", bufs=3, space="PSUM") as ps_b, \
             tc.tile_pool(name="ps_o", bufs=GT, space="PSUM") as ps_o:

            for g in range(NG):
                tiles_g = range(g * GT, (g + 1) * GT)
                h_tiles = {}
                for i in tiles_g:
                    qr = pb.tile([P, D], F32, tag="qB", name=f"qB{i}")
                    nc.sync.dma_start(qr[:], q_dr[i * P:(i + 1) * P, :])
                    kr = pb.tile([P, D], MMDT, tag="kB", name=f"kB{i}")
                    nc.sync.dma_start(kr[:], k_dr[i * P:(i + 1) * P, :])
                    kvr = pb.tile([P, D], MMDT, tag="kvB", name=f"kvB{i}")
                    nc.sync.dma_start(kvr[:], kv_dr[i * P:(i + 1) * P, :])
                    x_i = pb.tile([P, D], F32, tag="xB", name=f"xB{i}")
                    nc.sync.dma_start(x_i[:], x_dr[i * P:(i + 1) * P, :])

                    rk = pb.tile([P, D], F32, tag="rkB", name=f"rkB{i}")
                    asum = pb.tile([P, 1], F32, tag="asum", name=f"as{i}")
                    for (eo, ec) in cfg.d_chunks:
                        pck = ps_b.tile([P, ec], F32, tag="misc",
                                        name=f"pck{i}_{eo}")
                        nc.tensor.matmul(pck[:], uT_sb[:], kr[:, eo:eo + ec],
                                         start=True, stop=False)
                        nc.tensor.matmul(pck[:], ones1P_sb[:],
                                         offb_k[:, eo:eo + ec],
                                         start=False, stop=True)
                        nc.vector.reciprocal(rk[:, eo:eo + ec], pck[:])
                        pckv = ps_b.tile([P, ec], F32, tag="misc",
                                         name=f"pckv{i}_{eo}")
                        nc.tensor.matmul(pckv[:], uT_sb[:],
                                         kvr[:, eo:eo + ec],
                                         start=True, stop=False)
                        nc.tensor.matmul(pckv[:], ones1P_sb[:],
                                         offb_kv[:, eo:eo + ec],
                                         start=False, stop=True)
                        nc.vector.scalar_tensor_tensor(
                            kvr[:, eo:eo + ec], kvr[:, eo:eo + ec], 0.0,
                            pckv[:], op0=ALU.bypass, op1=ALU.add)
                    # advance the running offset rows for the next tile
                    if i < NT - 1:
                        for tots, off, offb, nm in (
                                (tots_k, off_k, offb_k, "k"),
                                (tots_kv, off_kv, offb_kv, "kv")):
                            trB = pb.tile([1, D], F32, tag="trB", bufs=2,
                                          name=f"trB{nm}{i}")
                            nc.sync.dma_start(trB[:], tots[i:i + 1, :])
                            nc.vector.tensor_tensor(off[:], off[:], trB[:],
                                                    op=ALU.add)
                            nc.vector.tensor_copy(offb[:], off[:])
                    nc.vector.tensor_tensor(kvr[:], kvr[:], rk[:],
                                            op=ALU.mult)
                    nc.vector.scalar_tensor_tensor(
                        qr[:], qr[:], 0.0, kvr[:], op0=ALU.bypass,
                        op1=ALU.mult, accum_out=asum[:])

                    # ---- LN1 ----
                    sq = pb.tile([P, D], F32, tag="sqB", name=f"sqB{i}",
                                 bufs=1)
                    s2 = pb.tile([P, 1], F32, tag="s2", name=f"s2_{i}")
                    nc.scalar.activation(sq[:], qr[:], AF.Square,
                                         accum_out=s2[:])
                    mean = pb.tile([P, 1], F32, tag="mean", name=f"mn{i}")
                    nc.vector.tensor_scalar(mean[:], asum[:], 1.0 / D, None,
                                            op0=ALU.mult)
                    m2 = pb.tile([P, 1], F32, tag="m2", name=f"m2_{i}")
                    nc.vector.tensor_tensor(m2[:], mean[:], mean[:],
                                            op=ALU.mult)
                    var = pb.tile([P, 1], F32, tag="var", name=f"vr{i}")
                    nc.vector.scalar_tensor_tensor(
                        var[:], s2[:], 1.0 / D, m2[:], op0=ALU.mult,
                        op1=ALU.subtract)
                    std = pb.tile([P, 1], F32, tag="std", name=f"sd{i}")
                    nc.scalar.activation(std[:], var[:], AF.Sqrt,
                                         bias=eps_sb[:])
                    inv = pb.tile([P, 1], F32, tag="inv", name=f"iv{i}")
                    nc.vector.reciprocal(inv[:], std[:])
                    nmi = pb.tile([P, 1], F32, tag="nmi", name=f"nm{i}")
                    nc.vector.scalar_tensor_tensor(
                        nmi[:], mean[:], -1.0, inv[:], op0=ALU.mult,
                        op1=ALU.mult)
                    nc.scalar.activation(qr[:], qr[:], AF.Identity,
                                         bias=nmi[:], scale=inv[:])
                    if not cfg.trivial_affine:
                        nc.vector.tensor_tensor(qr[:], qr[:],
                                                affine_sb["g1B"][:],
                                                op=ALU.mult)
                        nc.vector.tensor_tensor(qr[:], qr[:],
                                                affine_sb["be1B"][:],
                                                op=ALU.add)
                    h_i = hpool.tile([P, D], F32, tag="h", name=f"h{i}")
                    nc.vector.tensor_tensor(h_i[:], x_i[:], qr[:],
                                            op=ALU.add)
                    h_tiles[i] = h_i

                # ---- h -> hT ----
                TG = GT * P
                ht = htp.tile([P, KT * TG], MMDT, tag="ht", name=f"ht{g}")
                for kt in range(KT):
                    for jb in range(0, GT, 4):
                        nb = min(4, GT - jb)
                        pt = ps_tr.tile([P, nb * P], F32, tag="tr",
                                        name=f"ptrh{g}_{kt}_{jb}")
                        for z in range(nb):
                            i = g * GT + jb + z
                            nc.tensor.transpose(
                                pt[:, z * P:(z + 1) * P],
                                h_tiles[i][:, kt * P:(kt + 1) * P], id_sb[:])
                        nc.scalar.copy(
                            ht[:, kt * TG + jb * P: kt * TG + (jb + nb) * P],
                            pt[:])

                # ---- FFN1 ----
                f1t = f1p.tile([P, FB * TG], cfg.f1_dtype, tag="f1t",
                               name=f"f1t{g}")
                w1r = w1.rearrange("(kt p) f -> p kt f", p=P)
                for fb in range(FB):
                    w1t = w1p.tile([P, KT * P], MMDT, tag="w1t",
                                   name=f"w1t{g}_{fb}")
                    nc.sync.dma_start(
                        w1t[:].rearrange("p (kt f) -> p kt f", kt=KT),
                        w1r[:, :, fb * P:(fb + 1) * P])
                    pf = ps_b.tile([P, TG], F32, tag="misc",
                                   name=f"pf{g}_{fb}")
                    for kt in range(KT):
                        nc.tensor.matmul(
                            pf[:],
                            w1t[:, kt * P:(kt + 1) * P],
                            ht[:, kt * TG:(kt + 1) * TG],
                            start=(kt == 0), stop=(kt == KT - 1))
                    r_f = pb.tile([P, TG], F32, tag="rf", name=f"rf{g}_{fb}")
                    nc.scalar.activation(r_f[:], pf[:], AF.Relu,
                                         bias=b1c_sb[:, fb:fb + 1])
                    if fb % 2 == 0:
                        nc.scalar.square(f1t[:, fb * TG:(fb + 1) * TG],
                                         r_f[:])
                    else:
                        nc.vector.tensor_tensor(
                            f1t[:, fb * TG:(fb + 1) * TG], r_f[:], r_f[:],
                            op=ALU.mult)

                # ---- FFN2 ----
                f2_tiles = {}
                for i in tiles_g:
                    f2_tiles[i] = pb.tile([P, D], F32, tag="f2",
                                          name=f"f2_{i}", bufs=GT)
                f2sum = {i: [] for i in tiles_g}
                for (eo, ec) in cfg.d_chunks:
                    pos = {}
                    for i in tiles_g:
                        pos[i] = ps_o.tile([P, ec], F32, tag="po",
                                           name=f"pq{i}_{eo}")
                    for fb in range(FB):
                        w2t = w2p.tile([P, ec], cfg.w2_dtype, tag="w2t",
                                       name=f"w2t{g}_{eo}_{fb}")
                        nc.sync.dma_start(
                            w2t[:], w2[fb * P:(fb + 1) * P, eo:eo + ec])
                        for j, i in enumerate(tiles_g):
                            nc.tensor.matmul(
                                pos[i][:],
                                f1t[:, fb * TG + j * P:fb * TG + (j + 1) * P],
                                w2t[:], start=(fb == 0), stop=(fb == FB - 1))
                    for i in tiles_g:
                        acc = pb.tile([P, 1], F32, tag="f2s",
                                      name=f"f2s{i}_{eo}", bufs=2 * GT)
                        nc.vector.scalar_tensor_tensor(
                            f2_tiles[i][:, eo:eo + ec], pos[i][:], 0.0,
                            b2B_sb[:, eo:eo + ec], op0=ALU.bypass,
                            op1=ALU.add, accum_out=acc[:])
                        f2sum[i].append(acc)

                # ---- LN2 + residual ----
                for i in tiles_g:
                    f2_i = f2_tiles[i]
                    asum2 = pb.tile([P, 1], F32, tag="asum2",
                                    name=f"as2_{i}")
                    accs = f2sum[i]
                    if len(accs) == 1:
                        nc.vector.tensor_copy(asum2[:], accs[0][:])
                    else:
                        nc.vector.tensor_tensor(asum2[:], accs[0][:],
                                                accs[1][:], op=ALU.add)
                        for a in accs[2:]:
                            nc.vector.tensor_tensor(asum2[:], asum2[:],
                                                    a[:], op=ALU.add)
                    sq2 = pb.tile([P, D], F32, tag="sqB", name=f"sq2_{i}",
                                  bufs=1)
                    s22 = pb.tile([P, 1], F32, tag="s22", name=f"s22_{i}")
                    nc.scalar.activation(sq2[:], f2_i[:], AF.Square,
                                         accum_out=s22[:])
                    mean2 = pb.tile([P, 1], F32, tag="mean2",
                                    name=f"mn2_{i}")
                    nc.vector.tensor_scalar(mean2[:], asum2[:], 1.0 / D,
                                            None, op0=ALU.mult)
                    m22 = pb.tile([P, 1], F32, tag="m22", name=f"m22_{i}")
                    nc.vector.tensor_tensor(m22[:], mean2[:], mean2[:],
                                            op=ALU.mult)
                    var2 = pb.tile([P, 1], F32, tag="var2", name=f"vr2_{i}")
                    nc.vector.scalar_tensor_tensor(
                        var2[:], s22[:], 1.0 / D, m22[:], op0=ALU.mult,
                        op1=ALU.subtract)
                    std2 = pb.tile([P, 1], F32, tag="std2", name=f"sd2_{i}")
                    nc.scalar.activation(std2[:], var2[:], AF.Sqrt,
                                         bias=eps_sb[:])
                    inv2 = pb.tile([P, 1], F32, tag="inv2", name=f"iv2_{i}")
                    nc.vector.reciprocal(inv2[:], std2[:])
                    nmi2 = pb.tile([P, 1], F32, tag="nmi2", name=f"nm2_{i}")
                    nc.vector.scalar_tensor_tensor(
                        nmi2[:], mean2[:], -1.0, inv2[:], op0=ALU.mult,
                        op1=ALU.mult)
                    nc.scalar.activation(f2_i[:], f2_i[:], AF.Identity,
                                         bias=nmi2[:], scale=inv2[:])
                    if not cfg.trivial_affine:
                        nc.vector.tensor_tensor(f2_i[:], f2_i[:],
                                                affine_sb["g2B"][:],
                                                op=ALU.mult)
                        nc.vector.tensor_tensor(f2_i[:], f2_i[:],
                                                affine_sb["be2B"][:],
                                                op=ALU.add)
                    o_i = pb.tile([P, D], F32, tag="oB", name=f"oB{i}")
                    nc.vector.tensor_tensor(o_i[:], h_tiles[i][:], f2_i[:],
                                            op=ALU.add)
                    nc.sync.dma_start(out[i * P:(i + 1) * P, :], o_i[:])

    nc.finalize()
    return nc


# ------------------------------------------------------------------
# host wrapper
# ------------------------------------------------------------------

def _pmask(cfg, core):
    m = np.zeros((cfg.n_cores, 1), np.float32)
    if core % 2 == 1:
        m[core - 1, 0] = 1.0
    return m


def _prep_core_inputs(cfg: Cfg, inputs, core):
    np_bf16 = mybir.dt.np(BF16)
    b = core // 2
    half = core % 2
    T, D, NT = cfg.T, cfg.D, cfg.NT
    f32 = np.float32

    def a(x):
        return np.ascontiguousarray(np.asarray(x, dtype=f32))

    b1 = np.asarray(inputs["b1"], dtype=f32)
    b2 = np.asarray(inputs["b2"], dtype=f32)
    w2 = np.asarray(inputs["w2"], dtype=f32)
    if cfg.w2_dtype == BF16:
        w2 = w2.astype(np_bf16)
    mmnp = mybir.dt.np(cfg.mm_dtype)
    m = {
        "xe": a(inputs["x_enc"][b, half * T:(half + 1) * T, :]),
        "xp": a(inputs["x_pos"][b, half * T:(half + 1) * T, :]),
        "wq": a(inputs["wq"]).astype(mmnp),
        "wk": a(inputs["wk"]).astype(mmnp),
        "wv": a(inputs["wv"]).astype(mmnp),
        "w1": a(inputs["w1"]).astype(mmnp),
        "w2": np.ascontiguousarray(w2),
        "b1c": np.ascontiguousarray(b1.reshape(cfg.FB, P).T),
        "b2B": np.ascontiguousarray(np.broadcast_to(b2, (P, D)).copy()),
        "uT": np.triu(np.ones((P, P), f32)).astype(mmnp),
        "onesP1": np.ones((P, 1), mmnp),
        "ones1P": np.ones((1, P), mmnp),
        "ident": np.eye(P, dtype=f32),
        "pmask": _pmask(cfg, core),
    }
    if not cfg.trivial_affine:
        for nm, key in (("g1B", "g1"), ("be1B", "be1"), ("g2B", "g2"),
                        ("be2B", "be2")):
            m[nm] = np.ascontiguousarray(
                np.broadcast_to(np.asarray(inputs[key], f32), (P, D)).copy())
    return m


def run_kernel(inputs, cfg=None, trace=False):
    if cfg is None:
        cfg = Cfg()
    trivial = (np.all(np.asarray(inputs["g1"]) == 1.0)
               and np.all(np.asarray(inputs["be1"]) == 0.0)
               and np.all(np.asarray(inputs["g2"]) == 1.0)
               and np.all(np.asarray(inputs["be2"]) == 0.0))
    cfg.trivial_affine = bool(trivial)
    nc = build_nc(cfg)
    in_maps = [_prep_core_inputs(cfg, inputs, c) for c in range(cfg.n_cores)]
    res = run_bass_kernel_spmd(nc, in_maps, core_ids=list(range(cfg.n_cores)),
                               trace=trace)
    out = np.empty((cfg.B, cfg.S, cfg.D), np.float32)
    T = cfg.T
    for c in range(cfg.n_cores):
        out[c // 2, (c % 2) * T:((c % 2) + 1) * T, :] = res.results[c]["out"]
    return out, res


def kernel(**inputs):
    out, _ = run_kernel(inputs)
    return out
